# revision 21
# baseline (speedup 1.0000x reference)
"""Trainium2 Bass kernel for nn_AttentionBlock (B=4, C=512, T=2048, H=8, G=32).

Sharding: 8 cores = (batch b in 0..3) x (head-group hg in 0..1, 4 heads each).
Each core computes groupnorm(x[b]) (redundantly within the pair), its heads'
q/k/v, attention, and a partial projection using its head-group's w_proj
columns.  Host sums the two partials per batch; the hg==0 core folds in the
residual x and b_proj.

On-chip layout highlights:
 - all big matmuls use fp16 operands (fp32 streams the moving operand at
   ~2 cycles/element on the xbus; 2-byte dtypes stream at 1 — measured
   511ns vs ~215ns for N=512).  PSUM accumulation stays fp32.
 - QK^T computed in scoresT[s, t] layout; two heads of a pair occupy
   partition halves 0-63 / 64-127 so their K=64 matmuls run concurrently on
   distinct PE row-groups.
 - exp is split between the ACT engine (exact, AF.Exp) and the DVE
   (Schraudolph fast-exp: one tensor_scalar computing round(score*1477.32
   + 15315.25) into int16, whose bits ARE the fp16 approximation of
   exp(score), +-3% element error that largely cancels post-softmax).
 - V generated s-major; a constant ones column (memset once per tile) is
   appended per head so the PV matmul emits both out^T[d, t] and the
   softmax row sums.
 - k-bias is dropped entirely (softmax-invariant once q keeps its bias).
 - v-bias is folded into the projection bias on the host (exact).
 - softmax division: per-block reciprocal of the psum sums row (partition
   64) into an fp16 staging row, broadcast across partitions via a K=1
   ones matmul, one DVE multiply per 512-t block.
"""

import sys
from contextlib import ExitStack

sys.path.insert(0, "/opt/trn_rl_repo")

import numpy as np

import concourse.bass as bass
import concourse.tile as tile
from concourse import bacc, mybir
from concourse.bass_utils import run_bass_kernel_spmd

F32 = mybir.dt.float32
F16 = mybir.dt.float16
I16 = mybir.dt.int16
AF = mybir.ActivationFunctionType
OP = mybir.AluOpType

B, C, T = 4, 512, 2048
H = 8
DH = C // H          # 64
G = 32               # groupnorm groups
GS = C // G          # 16 channels per group
EPS = 1e-5
NKC = C // 128       # 4 c-chunks
SCALE = 1.0 / np.sqrt(np.sqrt(DH))

# Schraudolph fast-exp in fp16 bits: round(x*EXPA + EXPB) as int16 == fp16
# bits of ~exp(x); numerically optimized shift, max rel err 3.02%.
EXPA = 1477.3197218702985
EXPB = 15315.25
# sc-chunks routed to the DVE fast-exp (rest go to ACT exact exp)
DVE_EXP = (1, 3, 5, 7, 9, 11, 13)

_CACHE = {}


def build_program():
    nc = bacc.Bacc("TRN2", target_bir_lowering=False, debug=False)

    def inp(name, shape, dt=F32):
        return nc.dram_tensor(name, shape, dt, kind="ExternalInput").ap()

    x_d = inp("x", [C, T])
    wqk_d = inp("wqk", [C, 512], F16)
    wv_d = inp("wv", [C, 256], F16)
    wp_d = inp("wp", [256, 512], F16)
    smalls_d = inp("smalls", [128, 25])
    expander_d = inp("expander", [8, 128])
    y_d = nc.dram_tensor("y", [C, T], F32, kind="ExternalOutput").ap()

    x_r = x_d.rearrange("(k p) t -> k p t", p=128)
    y_r = y_d.rearrange("(k p) t -> k p t", p=128)

    with tile.TileContext(nc) as tc, ExitStack() as ctx:
        consts = ctx.enter_context(tc.tile_pool(name="consts", bufs=1))
        xpool = ctx.enter_context(tc.tile_pool(name="xpool", bufs=4))
        big = ctx.enter_context(tc.tile_pool(name="big", bufs=6))
        qkpool = ctx.enter_context(tc.tile_pool(name="qkpool", bufs=4))
        vtpool = ctx.enter_context(tc.tile_pool(name="vtpool", bufs=16))
        ptpool = ctx.enter_context(tc.tile_pool(name="ptpool", bufs=5))
        ysb = ctx.enter_context(tc.tile_pool(name="ysb", bufs=6))
        small = ctx.enter_context(tc.tile_pool(name="small", bufs=10))
        divp = ctx.enter_context(tc.tile_pool(name="divp", bufs=4))
        pp_sc = ctx.enter_context(tc.tile_pool(name="pp_sc", bufs=3, space="PSUM"))
        pp_out = ctx.enter_context(tc.tile_pool(name="pp_out", bufs=2, space="PSUM"))

        ctr = [0]

        def psum_sc():
            ctr[0] += 1
            return pp_sc.tile([128, 1024], F32, tag="sc", name=f"sc{ctr[0]}")

        def psum_out(width=512):
            ctr[0] += 1
            return pp_out.tile([128, width], F32, tag="po", name=f"po{ctr[0]}")

        # ---- DMA: weights early (qkgen needs wqk), x chunks in kc order,
        # dispatch spread over four engines so sequencer time doesn't
        # serialize the prologue ----
        xs = [
            xpool.tile([128, T], F32, tag="x", name=f"x{kc}") for kc in range(NKC)
        ]

        def xdma(eng, kc):
            for j in range(2):
                eng.dma_start(
                    out=xs[kc][:, j * 1024 : (j + 1) * 1024],
                    in_=x_r[kc][:, j * 1024 : (j + 1) * 1024],
                )

        wqk_sb = consts.tile([128, NKC, 512], F16)
        wv_sb = consts.tile([128, NKC, 256], F16)
        wp_sb = consts.tile([128, 2, 512], F16)
        smalls_sb = consts.tile([128, 25], F32)
        expander_sb = consts.tile([8, 128], F32)

        # tiny gn constants first (the per-kc gn chain blocks on them),
        # then wqk (first qkgen matmul), then x, then late-use weights
        nc.scalar.dma_start(out=smalls_sb, in_=smalls_d)
        nc.sync.dma_start(out=expander_sb, in_=expander_d)
        nc.gpsimd.dma_start(
            out=wqk_sb, in_=wqk_d.rearrange("(k p) m -> p k m", p=128)
        )
        xdma(nc.sync, 0)
        xdma(nc.gpsimd, 1)
        xdma(nc.sync, 2)
        xdma(nc.scalar, 3)
        nc.scalar.dma_start(
            out=wv_sb, in_=wv_d.rearrange("(k p) m -> p k m", p=128)
        )
        nc.sync.dma_start(
            out=wp_sb, in_=wp_d.rearrange("(k p) m -> p k m", p=128)
        )

        bqk_sb = smalls_sb[:, 0:4]
        bp_sb = smalls_sb[:, 4:8]
        rs_sb = smalls_sb[:, 8:9]
        gamma_sb = smalls_sb[:, 9:13]
        beta_sb = smalls_sb[:, 13:17]
        ones16_sb = smalls_sb[:, 17:25]
        eps_sb = consts.tile([128, 1], F32)
        nc.vector.memset(eps_sb, EPS)
        onesp_sb = consts.tile([65, 128], F16)
        nc.gpsimd.memset(onesp_sb, 1.0)

        # PE warmup: the HAM clock gate holds the array at 1.2 GHz until it
        # sees ~3.4us of sustained activity.  The x-load leaves the PE idle
        # for ~20us, so the first real matmul wave would run at half clock.
        # Stream dummy matmuls on a scratch tile during the DMA wait; they
        # finish before groupnorm needs the array.
        warm_sb = consts.tile([128, 512], F16)
        nc.vector.memset(warm_sb, 0.25)
        warm_ps = pp_sc.tile([128, 1024], F32, tag="sc", name="warm_ps")
        for _ in range(56):
            nc.tensor.matmul(
                warm_ps[:, 0:512],
                warm_sb[:, 0:128],
                warm_sb,
                start=True,
                stop=True,
            )

        # ---- groupnorm ----
        hs = []
        for kc in range(NKC):
            x_t = xs[kc]
            stats = small.tile([128, 4, 6], F32, tag="stats")
            for j in range(4):
                nc.vector.bn_stats(
                    out=stats[:, j, :], in_=x_t[:, j * 512 : (j + 1) * 512]
                )
            mv = small.tile([128, 2], F32, tag="mv")
            nc.vector.bn_aggr(out=mv, in_=stats)

            # pack rhs: col0 = mean_c, col1 = E[x^2]_c = var_c + mean_c^2
            pk = small.tile([128, 2], F32, tag="pk")
            nc.vector.tensor_copy(pk[:, 0:1], mv[:, 0:1])
            nc.vector.tensor_mul(pk[:, 1:2], mv[:, 0:1], mv[:, 0:1])
            nc.vector.tensor_add(pk[:, 1:2], pk[:, 1:2], mv[:, 1:2])

            ps_g = psum_out()
            nc.tensor.matmul(
                ps_g[0:8, 0:2], ones16_sb, pk, start=True, stop=True
            )
            # gm: col0 = mean_g, col1 = rstd_g
            gsum = small.tile([8, 2], F32, tag="gsum")
            nc.vector.tensor_copy(gsum, ps_g[0:8, 0:2])
            gm = small.tile([8, 2], F32, tag="gm")
            nc.vector.tensor_copy(gm[:, 0:1], gsum[:, 0:1])
            varg = small.tile([8, 1], F32, tag="varg")
            nc.vector.tensor_mul(varg, gsum[:, 0:1], gsum[:, 0:1])
            nc.vector.tensor_sub(varg, gsum[:, 1:2], varg)
            nc.scalar.activation(varg, varg, AF.Sqrt, bias=eps_sb[0:8, :])
            nc.vector.reciprocal(gm[:, 1:2], varg)

            ps_pc = psum_out()
            nc.tensor.matmul(
                ps_pc[0:128, 0:2], expander_sb, gm, start=True, stop=True
            )
            scale = small.tile([128, 1], F32, tag="scale")
            nc.vector.tensor_mul(scale, ps_pc[:, 1:2], gamma_sb[:, kc : kc + 1])
            nbias = small.tile([128, 1], F32, tag="nbias")
            nc.vector.tensor_mul(nbias, ps_pc[:, 0:1], scale)
            nc.vector.tensor_sub(nbias, beta_sb[:, kc : kc + 1], nbias)

            h_t = big.tile([128, T], F16, tag="big")
            if kc == 3:
                nc.vector.tensor_scalar(
                    out=h_t,
                    in0=x_t,
                    scalar1=scale,
                    scalar2=nbias,
                    op0=OP.mult,
                    op1=OP.add,
                )
            else:
                nc.scalar.activation(
                    h_t, x_t, AF.Identity, bias=nbias, scale=scale
                )
            hs.append(h_t)

        # ---- q/k generation: m-chunks [qP0, kP0, qP1, kP1].  kc-major so
        # the stationary wqk slice is loaded once per kc and reused by both
        # 512-t halves, and early-kc matmuls can run while later x chunks
        # are still loading. ----
        qk_tiles = [
            qkpool.tile([128, T], F16, tag="qk", name=f"qk{mc}") for mc in range(4)
        ]

        def gen_qk_part(mc, tc2):
            dest = qk_tiles[mc]
            ps = psum_sc()
            for kc in range(NKC):
                for half in range(2):
                    t0 = (tc2 * 2 + half) * 512
                    nc.tensor.matmul(
                        ps[:, half * 512 : half * 512 + 512],
                        wqk_sb[:, kc, mc * 128 : mc * 128 + 128],
                        hs[kc][:, t0 : t0 + 512],
                        start=(kc == 0),
                        stop=(kc == NKC - 1),
                    )
            if mc % 2 == 0:  # q chunks: add bias
                nc.vector.tensor_scalar(
                    out=dest[:, tc2 * 1024 : tc2 * 1024 + 1024],
                    in0=ps,
                    scalar1=bqk_sb[:, mc : mc + 1],
                    scalar2=None,
                    op0=OP.add,
                )
            else:  # k chunks: bias dropped (softmax-invariant)
                nc.scalar.activation(
                    dest[:, tc2 * 1024 : tc2 * 1024 + 1024], ps, AF.Copy
                )

        # ---- v generation, s-major, 4 sc-chunks per psum tile, kc-major;
        # ones column memset per head block ----
        vts = [None] * 16

        def vgen_group(g):
            # two sc-chunks per psum tile, one per 512-col bank: interleaved
            # accumulation groups must not share a PSUM bank (start clears
            # bank-wide state, not just the instruction's columns).
            for ii in range(2):
                ps = psum_sc()
                for kc in range(NKC):
                    for i in range(2):
                        sc = g * 4 + ii * 2 + i
                        nc.tensor.matmul(
                            ps[:, i * 512 : i * 512 + 256],
                            hs[kc][:, sc * 128 : sc * 128 + 128],
                            wv_sb[:, kc, :],
                            start=(kc == 0),
                            stop=(kc == NKC - 1),
                        )
                for i in range(2):
                    sc = g * 4 + ii * 2 + i
                    vt = vtpool.tile([128, 4, 65], F16, tag="vt")
                    nc.gpsimd.memset(vt[:, :, 64:65], 1.0)
                    nc.vector.tensor_copy(
                        vt[:, :, 0:64],
                        ps[:, i * 512 : i * 512 + 256].rearrange(
                            "p (h d) -> p h d", d=64
                        ),
                    )
                    vts[sc] = vt

        # pair-0 q/k and the first v-group up front; the rest is spliced
        # into early attention iterations (PE slack under the ACT-bound
        # loop) via the `after` hooks below.
        for mc in (0, 1):
            for tc2 in (0, 1):
                gen_qk_part(mc, tc2)
        vgen_group(0)
        qpair = [qk_tiles[0], qk_tiles[2]]
        kpair = [qk_tiles[1], qk_tiles[3]]

        def proj_tc(tc4):
            for mc in range(4):
                ps = psum_sc()
                for kc2 in range(2):
                    nc.tensor.matmul(
                        ps[:, 0:512],
                        wp_sb[:, kc2, mc * 128 : mc * 128 + 128],
                        att[kc2][:, tc4 * 512 : tc4 * 512 + 512],
                        start=(kc2 == 0),
                        stop=(kc2 == 1),
                    )
                xz = ysb.tile([128, 512], F32, tag="y")
                nc.gpsimd.tensor_scalar(
                    out=xz,
                    in0=xs[mc][:, tc4 * 512 : tc4 * 512 + 512],
                    scalar1=rs_sb,
                    scalar2=bp_sb[:, mc : mc + 1],
                    op0=OP.mult,
                    op1=OP.add,
                )
                yt = ysb.tile([128, 512], F32, tag="y")
                nc.vector.tensor_add(yt, ps[:, 0:512], xz)
                eng = nc.sync if (mc + tc4) % 2 == 0 else nc.gpsimd
                eng.dma_start(
                    out=y_r[mc][:, tc4 * 512 : tc4 * 512 + 512], in_=yt
                )

        # ---- attention ----
        att = [big.tile([128, T], F16, tag="big", name=f"att{i}") for i in range(2)]
        rrs = {}

        def divide_tq(pr, tq):
            t0 = tq * 512
            rr = rrs.pop((pr, tq))
            bc = psum_sc()
            for hip in range(2):
                nc.tensor.matmul(
                    bc[hip * 64 : hip * 64 + 64, 0:512],
                    onesp_sb[64:65, 0:64],
                    rr[64:65, hip * 512 : hip * 512 + 512],
                    start=True,
                    stop=True,
                )
            rb = divp.tile([128, 512], F32, tag="rb", name=f"rb{pr}{tq}")
            nc.vector.reciprocal_approx_fast(out=rb, in_=bc[:, 0:512])
            a_slc = att[pr][:, t0 : t0 + 512]
            nc.vector.tensor_mul(a_slc, a_slc, rb)

        def emit_qk(pr, tq, sc):
            qp, kp = qpair[pr], kpair[pr]
            t0 = tq * 512
            ps = psum_sc()
            nc.tensor.matmul(
                ps[:, 0:512],
                kp[0:64, sc * 128 : sc * 128 + 128],
                qp[0:64, t0 : t0 + 512],
                start=True,
                stop=True,
            )
            nc.tensor.matmul(
                ps[:, 512:1024],
                kp[64:128, sc * 128 : sc * 128 + 128],
                qp[64:128, t0 : t0 + 512],
                start=True,
                stop=True,
            )
            return ps

        def emit_pv(pr, tq, sc, ps, outA, outB):
            pt_t = ptpool.tile([128, 1024], F16, tag="pt")
            if sc in DVE_EXP:
                nc.vector.tensor_scalar(
                    out=pt_t.bitcast(I16),
                    in0=ps,
                    scalar1=EXPA,
                    scalar2=EXPB,
                    op0=OP.mult,
                    op1=OP.add,
                )
            else:
                nc.scalar.activation(pt_t, ps, AF.Exp)
            va = vts[sc][:, pr * 2 + 0, 0:65]
            vb = vts[sc][:, pr * 2 + 1, 0:65]
            nc.tensor.matmul(
                outA[0:65, 0:512],
                va,
                pt_t[:, 0:512],
                start=(sc == 0),
                stop=(sc == 15),
            )
            nc.tensor.matmul(
                outB[0:65, 0:512],
                vb,
                pt_t[:, 512:1024],
                start=(sc == 0),
                stop=(sc == 15),
            )

        def block_epilogue(pr, tq, outA, outB):
            t0 = tq * 512
            rr = divp.tile([65, 1024], F16, tag="rr", name=f"rr{pr}_{tq}")
            rrs[(pr, tq)] = rr
            for hip, outp in ((0, outA), (1, outB)):
                nc.vector.tensor_copy(
                    att[pr][hip * 64 : hip * 64 + 64, t0 : t0 + 512],
                    outp[0:64, :],
                )
                nc.scalar.activation(
                    rr[64:65, hip * 512 : hip * 512 + 512],
                    outp[64:65, 0:512],
                    AF.Copy,
                )

        # flattened attention iterations with one-deep QK lookahead so the
        # PE issues QK(i+1) while ACT runs exp(i) — keeps both engines
        # back-to-back.  Division quarters / projection chunks are spliced
        # between iterations once their inputs are long since staged.
        iters = [(pr, tq, sc) for pr in range(2) for tq in range(4) for sc in range(16)]
        after = {
            (0, 0, 0): lambda: vgen_group(1),
            (0, 0, 3): lambda: vgen_group(2),
            (0, 0, 6): lambda: vgen_group(3),
            (0, 0, 9): lambda: gen_qk_part(2, 0),
            (0, 0, 12): lambda: gen_qk_part(2, 1),
            (0, 1, 0): lambda: gen_qk_part(3, 0),
            (0, 1, 3): lambda: gen_qk_part(3, 1),
            (0, 1, 8): lambda: divide_tq(0, 0),
            (0, 2, 8): lambda: divide_tq(0, 1),
            (0, 3, 8): lambda: divide_tq(0, 2),
            (1, 0, 8): lambda: divide_tq(0, 3),
            (1, 1, 8): lambda: divide_tq(1, 0),
            (1, 1, 12): lambda: proj_tc(0),
            (1, 2, 8): lambda: divide_tq(1, 1),
            (1, 2, 12): lambda: proj_tc(1),
            (1, 3, 8): lambda: divide_tq(1, 2),
            (1, 3, 12): lambda: proj_tc(2),
        }
        outs = {}
        ps_next = emit_qk(*iters[0])
        for i, (pr, tq, sc) in enumerate(iters):
            if sc == 0:
                outs[(pr, tq)] = (psum_out(), psum_out())
            ps_cur = ps_next
            if i + 1 < len(iters):
                ps_next = emit_qk(*iters[i + 1])
            outA, outB = outs[(pr, tq)]
            emit_pv(pr, tq, sc, ps_cur, outA, outB)
            if sc == 15:
                block_epilogue(pr, tq, outA, outB)
                del outs[(pr, tq)]
            hook = after.get((pr, tq, sc))
            if hook is not None:
                hook()
        divide_tq(1, 3)
        proj_tc(3)

    nc.compile()
    return nc


def _consts():
    expander = np.zeros((8, 128), np.float32)
    for g in range(8):
        expander[g, g * 16 : (g + 1) * 16] = 1.0
    return expander


def _core_weights(hg, w_qkv, b_qkv, w_proj, b_proj, gn_gamma, gn_beta):
    heads = [4 * hg + i for i in range(4)]
    qrows, krows, vrows = [], [], []
    for h in heads:
        base = h * 3 * DH
        qrows.append(np.arange(base, base + DH))
        krows.append(np.arange(base + DH, base + 2 * DH))
        vrows.append(np.arange(base + 2 * DH, base + 3 * DH))
    # m-chunks: [qP0, kP0, qP1, kP1]; each pair chunk = [head_even | head_odd]
    qk_order = np.concatenate(
        [qrows[0], qrows[1], krows[0], krows[1], qrows[2], qrows[3], krows[2], krows[3]]
    )
    wqk = (w_qkv[qk_order].T * SCALE).astype(np.float16)
    bqk = np.ascontiguousarray((b_qkv[qk_order] * SCALE).reshape(4, 128).T)
    # v weights: [C, 4 heads, 64]; ones column appended on-chip via memset
    vrows_cat = np.concatenate(vrows)
    wv = np.ascontiguousarray(w_qkv[vrows_cat].T).astype(np.float16)
    att_cols = np.concatenate([np.arange(h * DH, (h + 1) * DH) for h in heads])
    wp = (w_proj[:, att_cols].T).astype(np.float16)
    # v-bias folded into projection bias: wp.T @ bv is this head-group's
    # constant contribution to every output column (exact for any b_qkv).
    bv = b_qkv[vrows_cat]  # (256,)
    bp_fold = w_proj[:, att_cols] @ bv  # (512,)
    if hg == 0:
        bp = np.ascontiguousarray((b_proj + bp_fold).reshape(4, 128).T)
        rs = np.ones((128, 1), np.float32)
    else:
        bp = np.ascontiguousarray(bp_fold.reshape(4, 128).T)
        rs = np.zeros((128, 1), np.float32)
    gamma = np.ascontiguousarray(gn_gamma.reshape(4, 128).T)
    beta = np.ascontiguousarray(gn_beta.reshape(4, 128).T)
    ones16 = np.zeros((128, 8), np.float32)
    for g in range(8):
        ones16[g * 16 : (g + 1) * 16, g] = 1.0 / GS
    smalls = np.concatenate([bqk, bp, rs, gamma, beta, ones16], axis=1)
    return dict(wqk=wqk, wv=wv, wp=wp, smalls=smalls.astype(np.float32))


def kernel(x, gn_gamma, gn_beta, w_qkv, b_qkv, w_proj, b_proj, _trace=False):
    x = np.asarray(x, np.float32)
    gn_gamma = np.asarray(gn_gamma, np.float32)
    gn_beta = np.asarray(gn_beta, np.float32)
    w_qkv = np.asarray(w_qkv, np.float32)
    b_qkv = np.asarray(b_qkv, np.float32)
    w_proj = np.asarray(w_proj, np.float32)
    b_proj = np.asarray(b_proj, np.float32)

    if "nc" not in _CACHE:
        _CACHE["nc"] = build_program()
    nc = _CACHE["nc"]

    expander = _consts()
    hg_consts = [
        _core_weights(hg, w_qkv, b_qkv, w_proj, b_proj, gn_gamma, gn_beta)
        for hg in range(2)
    ]
    in_maps = []
    for core in range(8):
        b, hg = core // 2, core % 2
        m = dict(hg_consts[hg])
        m["x"] = np.ascontiguousarray(x[b])
        m["expander"] = expander
        in_maps.append(m)

    res = run_bass_kernel_spmd(
        nc, in_maps, core_ids=list(range(8)), trace=_trace
    )
    y = np.empty((B, C, T), np.float32)
    for b in range(B):
        y[b] = res.results[2 * b]["y"] + res.results[2 * b + 1]["y"]
    if _trace:
        _CACHE["last_results"] = res
    return y


# revision 22
# speedup vs baseline: 1.0199x; 1.0199x over previous
"""Trainium2 Bass kernel for nn_AttentionBlock (B=4, C=512, T=2048, H=8, G=32).

Sharding: 8 cores = (batch b in 0..3) x (head-group hg in 0..1, 4 heads each).
Each core computes groupnorm(x[b]) (redundantly within the pair), its heads'
q/k/v, attention, and a partial projection using its head-group's w_proj
columns.  Host sums the two partials per batch; the hg==0 core folds in the
residual x and b_proj.

On-chip layout highlights:
 - all big matmuls use fp16 operands (fp32 streams the moving operand at
   ~2 cycles/element on the xbus; 2-byte dtypes stream at 1 — measured
   511ns vs ~215ns for N=512).  PSUM accumulation stays fp32.
 - QK^T computed in scoresT[s, t] layout; two heads of a pair occupy
   partition halves 0-63 / 64-127 so their K=64 matmuls run concurrently on
   distinct PE row-groups.
 - exp is split between the ACT engine (exact, AF.Exp) and the DVE
   (Schraudolph fast-exp: one tensor_scalar computing round(score*1477.32
   + 15315.25) into int16, whose bits ARE the fp16 approximation of
   exp(score), +-3% element error that largely cancels post-softmax).
 - V generated s-major; a constant ones column (memset once per tile) is
   appended per head so the PV matmul emits both out^T[d, t] and the
   softmax row sums.
 - k-bias is dropped entirely (softmax-invariant once q keeps its bias).
 - v-bias is folded into the projection bias on the host (exact).
 - softmax division: per-block reciprocal of the psum sums row (partition
   64) into an fp16 staging row, broadcast across partitions via a K=1
   ones matmul, one DVE multiply per 512-t block.
"""

import sys
from contextlib import ExitStack

sys.path.insert(0, "/opt/trn_rl_repo")

import numpy as np

import concourse.bass as bass
import concourse.tile as tile
from concourse import bacc, mybir
from concourse.bass_utils import run_bass_kernel_spmd

F32 = mybir.dt.float32
F16 = mybir.dt.float16
I16 = mybir.dt.int16
AF = mybir.ActivationFunctionType
OP = mybir.AluOpType

B, C, T = 4, 512, 2048
H = 8
DH = C // H          # 64
G = 32               # groupnorm groups
GS = C // G          # 16 channels per group
EPS = 1e-5
NKC = C // 128       # 4 c-chunks
SCALE = 1.0 / np.sqrt(np.sqrt(DH))

# Schraudolph fast-exp in fp16 bits: round(x*EXPA + EXPB) as int16 == fp16
# bits of ~exp(x); numerically optimized shift, max rel err 3.02%.
EXPA = 1477.3197218702985
EXPB = 15315.25
# sc-chunks routed to the DVE fast-exp (rest go to ACT exact exp)
DVE_EXP = (1, 3, 5, 7, 9, 11, 13)

_CACHE = {}


def build_program():
    nc = bacc.Bacc("TRN2", target_bir_lowering=False, debug=False)

    def inp(name, shape, dt=F32):
        return nc.dram_tensor(name, shape, dt, kind="ExternalInput").ap()

    x_d = inp("x", [C, T])
    wqk_d = inp("wqk", [C, 512], F16)
    wv_d = inp("wv", [C, 256], F16)
    wp_d = inp("wp", [256, 512], F16)
    smalls_d = inp("smalls", [128, 25])
    expander_d = inp("expander", [8, 128])
    y_d = nc.dram_tensor("y", [C, T], F32, kind="ExternalOutput").ap()

    x_r = x_d.rearrange("(k p) t -> k p t", p=128)
    y_r = y_d.rearrange("(k p) t -> k p t", p=128)

    with tile.TileContext(nc) as tc, ExitStack() as ctx:
        consts = ctx.enter_context(tc.tile_pool(name="consts", bufs=1))
        xpool = ctx.enter_context(tc.tile_pool(name="xpool", bufs=4))
        big = ctx.enter_context(tc.tile_pool(name="big", bufs=6))
        qkpool = ctx.enter_context(tc.tile_pool(name="qkpool", bufs=4))
        vtpool = ctx.enter_context(tc.tile_pool(name="vtpool", bufs=16))
        ptpool = ctx.enter_context(tc.tile_pool(name="ptpool", bufs=5))
        ysb = ctx.enter_context(tc.tile_pool(name="ysb", bufs=6))
        small = ctx.enter_context(tc.tile_pool(name="small", bufs=10))
        divp = ctx.enter_context(tc.tile_pool(name="divp", bufs=4))
        pp_sc = ctx.enter_context(tc.tile_pool(name="pp_sc", bufs=3, space="PSUM"))
        pp_out = ctx.enter_context(tc.tile_pool(name="pp_out", bufs=2, space="PSUM"))

        ctr = [0]

        def psum_sc():
            ctr[0] += 1
            return pp_sc.tile([128, 1024], F32, tag="sc", name=f"sc{ctr[0]}")

        def psum_out(width=512):
            ctr[0] += 1
            return pp_out.tile([128, width], F32, tag="po", name=f"po{ctr[0]}")

        # ---- DMA: weights early (qkgen needs wqk), x chunks in kc order,
        # dispatch spread over four engines so sequencer time doesn't
        # serialize the prologue ----
        xs = [
            xpool.tile([128, T], F32, tag="x", name=f"x{kc}") for kc in range(NKC)
        ]

        def xdma(eng, kc):
            for j in range(2):
                eng.dma_start(
                    out=xs[kc][:, j * 1024 : (j + 1) * 1024],
                    in_=x_r[kc][:, j * 1024 : (j + 1) * 1024],
                )

        wqk_sb = consts.tile([128, NKC, 512], F16)
        wv_sb = consts.tile([128, NKC, 256], F16)
        wp_sb = consts.tile([128, 2, 512], F16)
        smalls_sb = consts.tile([128, 25], F32)
        expander_sb = consts.tile([8, 128], F32)

        # tiny gn constants first (the per-kc gn chain blocks on them),
        # then wqk (first qkgen matmul), then x, then late-use weights
        nc.scalar.dma_start(out=smalls_sb, in_=smalls_d)
        nc.sync.dma_start(out=expander_sb, in_=expander_d)
        nc.gpsimd.dma_start(
            out=wqk_sb, in_=wqk_d.rearrange("(k p) m -> p k m", p=128)
        )
        xdma(nc.sync, 0)
        xdma(nc.gpsimd, 1)
        xdma(nc.sync, 2)
        xdma(nc.scalar, 3)
        nc.scalar.dma_start(
            out=wv_sb, in_=wv_d.rearrange("(k p) m -> p k m", p=128)
        )
        nc.sync.dma_start(
            out=wp_sb, in_=wp_d.rearrange("(k p) m -> p k m", p=128)
        )

        bqk_sb = smalls_sb[:, 0:4]
        bp_sb = smalls_sb[:, 4:8]
        rs_sb = smalls_sb[:, 8:9]
        gamma_sb = smalls_sb[:, 9:13]
        beta_sb = smalls_sb[:, 13:17]
        ones16_sb = smalls_sb[:, 17:25]
        eps_sb = consts.tile([128, 1], F32)
        nc.vector.memset(eps_sb, EPS)
        onesp_sb = consts.tile([65, 128], F16)
        nc.gpsimd.memset(onesp_sb, 1.0)

        # ---- groupnorm ----
        hs = []
        for kc in range(NKC):
            x_t = xs[kc]
            stats = small.tile([128, 4, 6], F32, tag="stats")
            for j in range(4):
                nc.vector.bn_stats(
                    out=stats[:, j, :], in_=x_t[:, j * 512 : (j + 1) * 512]
                )
            mv = small.tile([128, 2], F32, tag="mv")
            nc.vector.bn_aggr(out=mv, in_=stats)

            # pack rhs: col0 = mean_c, col1 = E[x^2]_c = var_c + mean_c^2
            pk = small.tile([128, 2], F32, tag="pk")
            nc.vector.tensor_copy(pk[:, 0:1], mv[:, 0:1])
            nc.vector.tensor_mul(pk[:, 1:2], mv[:, 0:1], mv[:, 0:1])
            nc.vector.tensor_add(pk[:, 1:2], pk[:, 1:2], mv[:, 1:2])

            ps_g = psum_out()
            nc.tensor.matmul(
                ps_g[0:8, 0:2], ones16_sb, pk, start=True, stop=True
            )
            # gm: col0 = mean_g, col1 = rstd_g
            gsum = small.tile([8, 2], F32, tag="gsum")
            nc.vector.tensor_copy(gsum, ps_g[0:8, 0:2])
            gm = small.tile([8, 2], F32, tag="gm")
            nc.vector.tensor_copy(gm[:, 0:1], gsum[:, 0:1])
            varg = small.tile([8, 1], F32, tag="varg")
            nc.vector.tensor_mul(varg, gsum[:, 0:1], gsum[:, 0:1])
            nc.vector.tensor_sub(varg, gsum[:, 1:2], varg)
            nc.scalar.activation(varg, varg, AF.Sqrt, bias=eps_sb[0:8, :])
            nc.vector.reciprocal(gm[:, 1:2], varg)

            ps_pc = psum_out()
            nc.tensor.matmul(
                ps_pc[0:128, 0:2], expander_sb, gm, start=True, stop=True
            )
            scale = small.tile([128, 1], F32, tag="scale")
            nc.vector.tensor_mul(scale, ps_pc[:, 1:2], gamma_sb[:, kc : kc + 1])
            nbias = small.tile([128, 1], F32, tag="nbias")
            nc.vector.tensor_mul(nbias, ps_pc[:, 0:1], scale)
            nc.vector.tensor_sub(nbias, beta_sb[:, kc : kc + 1], nbias)

            h_t = big.tile([128, T], F16, tag="big")
            if kc == 3:
                nc.vector.tensor_scalar(
                    out=h_t,
                    in0=x_t,
                    scalar1=scale,
                    scalar2=nbias,
                    op0=OP.mult,
                    op1=OP.add,
                )
            else:
                nc.scalar.activation(
                    h_t, x_t, AF.Identity, bias=nbias, scale=scale
                )
            hs.append(h_t)

        # ---- q/k generation: m-chunks [qP0, kP0, qP1, kP1].  kc-major so
        # the stationary wqk slice is loaded once per kc and reused by both
        # 512-t halves, and early-kc matmuls can run while later x chunks
        # are still loading. ----
        qk_tiles = [
            qkpool.tile([128, T], F16, tag="qk", name=f"qk{mc}") for mc in range(4)
        ]

        def gen_qk_part(mc, tc2):
            dest = qk_tiles[mc]
            ps = psum_sc()
            for kc in range(NKC):
                for half in range(2):
                    t0 = (tc2 * 2 + half) * 512
                    nc.tensor.matmul(
                        ps[:, half * 512 : half * 512 + 512],
                        wqk_sb[:, kc, mc * 128 : mc * 128 + 128],
                        hs[kc][:, t0 : t0 + 512],
                        start=(kc == 0),
                        stop=(kc == NKC - 1),
                    )
            if mc % 2 == 0:  # q chunks: add bias
                nc.vector.tensor_scalar(
                    out=dest[:, tc2 * 1024 : tc2 * 1024 + 1024],
                    in0=ps,
                    scalar1=bqk_sb[:, mc : mc + 1],
                    scalar2=None,
                    op0=OP.add,
                )
            else:  # k chunks: bias dropped (softmax-invariant)
                nc.scalar.activation(
                    dest[:, tc2 * 1024 : tc2 * 1024 + 1024], ps, AF.Copy
                )

        # ---- v generation, s-major, 4 sc-chunks per psum tile, kc-major;
        # ones column memset per head block ----
        vts = [None] * 16

        def vgen_group(g):
            # two sc-chunks per psum tile, one per 512-col bank: interleaved
            # accumulation groups must not share a PSUM bank (start clears
            # bank-wide state, not just the instruction's columns).
            for ii in range(2):
                ps = psum_sc()
                for kc in range(NKC):
                    for i in range(2):
                        sc = g * 4 + ii * 2 + i
                        nc.tensor.matmul(
                            ps[:, i * 512 : i * 512 + 256],
                            hs[kc][:, sc * 128 : sc * 128 + 128],
                            wv_sb[:, kc, :],
                            start=(kc == 0),
                            stop=(kc == NKC - 1),
                        )
                for i in range(2):
                    sc = g * 4 + ii * 2 + i
                    vt = vtpool.tile([128, 4, 65], F16, tag="vt")
                    nc.gpsimd.memset(vt[:, :, 64:65], 1.0)
                    nc.vector.tensor_copy(
                        vt[:, :, 0:64],
                        ps[:, i * 512 : i * 512 + 256].rearrange(
                            "p (h d) -> p h d", d=64
                        ),
                    )
                    vts[sc] = vt

        # pair-0 q/k and the first v-group up front; the rest is spliced
        # into early attention iterations (PE slack under the ACT-bound
        # loop) via the `after` hooks below.
        for mc in (0, 1):
            for tc2 in (0, 1):
                gen_qk_part(mc, tc2)
        vgen_group(0)
        qpair = [qk_tiles[0], qk_tiles[2]]
        kpair = [qk_tiles[1], qk_tiles[3]]

        def proj_tc(tc4):
            for mc in range(4):
                ps = psum_sc()
                for kc2 in range(2):
                    nc.tensor.matmul(
                        ps[:, 0:512],
                        wp_sb[:, kc2, mc * 128 : mc * 128 + 128],
                        att[kc2][:, tc4 * 512 : tc4 * 512 + 512],
                        start=(kc2 == 0),
                        stop=(kc2 == 1),
                    )
                xz = ysb.tile([128, 512], F32, tag="y")
                nc.gpsimd.tensor_scalar(
                    out=xz,
                    in0=xs[mc][:, tc4 * 512 : tc4 * 512 + 512],
                    scalar1=rs_sb,
                    scalar2=bp_sb[:, mc : mc + 1],
                    op0=OP.mult,
                    op1=OP.add,
                )
                yt = ysb.tile([128, 512], F32, tag="y")
                nc.vector.tensor_add(yt, ps[:, 0:512], xz)
                eng = nc.sync if (mc + tc4) % 2 == 0 else nc.gpsimd
                eng.dma_start(
                    out=y_r[mc][:, tc4 * 512 : tc4 * 512 + 512], in_=yt
                )

        # ---- attention ----
        att = [big.tile([128, T], F16, tag="big", name=f"att{i}") for i in range(2)]
        rrs = {}

        def divide_tq(pr, tq):
            t0 = tq * 512
            rr = rrs.pop((pr, tq))
            bc = psum_sc()
            for hip in range(2):
                nc.tensor.matmul(
                    bc[hip * 64 : hip * 64 + 64, 0:512],
                    onesp_sb[64:65, 0:64],
                    rr[64:65, hip * 512 : hip * 512 + 512],
                    start=True,
                    stop=True,
                )
            rb = divp.tile([128, 512], F32, tag="rb", name=f"rb{pr}{tq}")
            nc.vector.reciprocal_approx_fast(out=rb, in_=bc[:, 0:512])
            a_slc = att[pr][:, t0 : t0 + 512]
            nc.vector.tensor_mul(a_slc, a_slc, rb)

        def emit_qk(pr, tq, sc):
            qp, kp = qpair[pr], kpair[pr]
            t0 = tq * 512
            ps = psum_sc()
            nc.tensor.matmul(
                ps[:, 0:512],
                kp[0:64, sc * 128 : sc * 128 + 128],
                qp[0:64, t0 : t0 + 512],
                start=True,
                stop=True,
            )
            nc.tensor.matmul(
                ps[:, 512:1024],
                kp[64:128, sc * 128 : sc * 128 + 128],
                qp[64:128, t0 : t0 + 512],
                start=True,
                stop=True,
            )
            return ps

        def emit_pv(pr, tq, sc, ps, outA, outB):
            pt_t = ptpool.tile([128, 1024], F16, tag="pt")
            if sc in DVE_EXP:
                nc.vector.tensor_scalar(
                    out=pt_t.bitcast(I16),
                    in0=ps,
                    scalar1=EXPA,
                    scalar2=EXPB,
                    op0=OP.mult,
                    op1=OP.add,
                )
            else:
                nc.scalar.activation(pt_t, ps, AF.Exp)
            va = vts[sc][:, pr * 2 + 0, 0:65]
            vb = vts[sc][:, pr * 2 + 1, 0:65]
            nc.tensor.matmul(
                outA[0:65, 0:512],
                va,
                pt_t[:, 0:512],
                start=(sc == 0),
                stop=(sc == 15),
            )
            nc.tensor.matmul(
                outB[0:65, 0:512],
                vb,
                pt_t[:, 512:1024],
                start=(sc == 0),
                stop=(sc == 15),
            )

        def block_epilogue(pr, tq, outA, outB):
            t0 = tq * 512
            rr = divp.tile([65, 1024], F16, tag="rr", name=f"rr{pr}_{tq}")
            rrs[(pr, tq)] = rr
            for hip, outp in ((0, outA), (1, outB)):
                nc.vector.tensor_copy(
                    att[pr][hip * 64 : hip * 64 + 64, t0 : t0 + 512],
                    outp[0:64, :],
                )
                nc.scalar.activation(
                    rr[64:65, hip * 512 : hip * 512 + 512],
                    outp[64:65, 0:512],
                    AF.Copy,
                )

        # flattened attention iterations with one-deep QK lookahead so the
        # PE issues QK(i+1) while ACT runs exp(i) — keeps both engines
        # back-to-back.  Division quarters / projection chunks are spliced
        # between iterations once their inputs are long since staged.
        iters = [(pr, tq, sc) for pr in range(2) for tq in range(4) for sc in range(16)]
        after = {
            (0, 0, 0): lambda: vgen_group(1),
            (0, 0, 3): lambda: vgen_group(2),
            (0, 0, 6): lambda: vgen_group(3),
            (0, 0, 9): lambda: gen_qk_part(2, 0),
            (0, 0, 12): lambda: gen_qk_part(2, 1),
            (0, 1, 0): lambda: gen_qk_part(3, 0),
            (0, 1, 3): lambda: gen_qk_part(3, 1),
            (0, 1, 8): lambda: divide_tq(0, 0),
            (0, 2, 8): lambda: divide_tq(0, 1),
            (0, 3, 8): lambda: divide_tq(0, 2),
            (1, 0, 8): lambda: divide_tq(0, 3),
            (1, 1, 8): lambda: divide_tq(1, 0),
            (1, 1, 12): lambda: proj_tc(0),
            (1, 2, 8): lambda: divide_tq(1, 1),
            (1, 2, 12): lambda: proj_tc(1),
            (1, 3, 8): lambda: divide_tq(1, 2),
            (1, 3, 12): lambda: proj_tc(2),
        }
        outs = {}
        ps_next = emit_qk(*iters[0])
        for i, (pr, tq, sc) in enumerate(iters):
            if sc == 0:
                outs[(pr, tq)] = (psum_out(), psum_out())
            ps_cur = ps_next
            if i + 1 < len(iters):
                ps_next = emit_qk(*iters[i + 1])
            outA, outB = outs[(pr, tq)]
            emit_pv(pr, tq, sc, ps_cur, outA, outB)
            if sc == 15:
                block_epilogue(pr, tq, outA, outB)
                del outs[(pr, tq)]
            hook = after.get((pr, tq, sc))
            if hook is not None:
                hook()
        divide_tq(1, 3)
        proj_tc(3)

    nc.compile()
    return nc


def _consts():
    expander = np.zeros((8, 128), np.float32)
    for g in range(8):
        expander[g, g * 16 : (g + 1) * 16] = 1.0
    return expander


def _core_weights(hg, w_qkv, b_qkv, w_proj, b_proj, gn_gamma, gn_beta):
    heads = [4 * hg + i for i in range(4)]
    qrows, krows, vrows = [], [], []
    for h in heads:
        base = h * 3 * DH
        qrows.append(np.arange(base, base + DH))
        krows.append(np.arange(base + DH, base + 2 * DH))
        vrows.append(np.arange(base + 2 * DH, base + 3 * DH))
    # m-chunks: [qP0, kP0, qP1, kP1]; each pair chunk = [head_even | head_odd]
    qk_order = np.concatenate(
        [qrows[0], qrows[1], krows[0], krows[1], qrows[2], qrows[3], krows[2], krows[3]]
    )
    wqk = (w_qkv[qk_order].T * SCALE).astype(np.float16)
    bqk = np.ascontiguousarray((b_qkv[qk_order] * SCALE).reshape(4, 128).T)
    # v weights: [C, 4 heads, 64]; ones column appended on-chip via memset
    vrows_cat = np.concatenate(vrows)
    wv = np.ascontiguousarray(w_qkv[vrows_cat].T).astype(np.float16)
    att_cols = np.concatenate([np.arange(h * DH, (h + 1) * DH) for h in heads])
    wp = (w_proj[:, att_cols].T).astype(np.float16)
    # v-bias folded into projection bias: wp.T @ bv is this head-group's
    # constant contribution to every output column (exact for any b_qkv).
    bv = b_qkv[vrows_cat]  # (256,)
    bp_fold = w_proj[:, att_cols] @ bv  # (512,)
    if hg == 0:
        bp = np.ascontiguousarray((b_proj + bp_fold).reshape(4, 128).T)
        rs = np.ones((128, 1), np.float32)
    else:
        bp = np.ascontiguousarray(bp_fold.reshape(4, 128).T)
        rs = np.zeros((128, 1), np.float32)
    gamma = np.ascontiguousarray(gn_gamma.reshape(4, 128).T)
    beta = np.ascontiguousarray(gn_beta.reshape(4, 128).T)
    ones16 = np.zeros((128, 8), np.float32)
    for g in range(8):
        ones16[g * 16 : (g + 1) * 16, g] = 1.0 / GS
    smalls = np.concatenate([bqk, bp, rs, gamma, beta, ones16], axis=1)
    return dict(wqk=wqk, wv=wv, wp=wp, smalls=smalls.astype(np.float32))


def kernel(x, gn_gamma, gn_beta, w_qkv, b_qkv, w_proj, b_proj, _trace=False):
    x = np.asarray(x, np.float32)
    gn_gamma = np.asarray(gn_gamma, np.float32)
    gn_beta = np.asarray(gn_beta, np.float32)
    w_qkv = np.asarray(w_qkv, np.float32)
    b_qkv = np.asarray(b_qkv, np.float32)
    w_proj = np.asarray(w_proj, np.float32)
    b_proj = np.asarray(b_proj, np.float32)

    if "nc" not in _CACHE:
        _CACHE["nc"] = build_program()
    nc = _CACHE["nc"]

    expander = _consts()
    hg_consts = [
        _core_weights(hg, w_qkv, b_qkv, w_proj, b_proj, gn_gamma, gn_beta)
        for hg in range(2)
    ]
    in_maps = []
    for core in range(8):
        b, hg = core // 2, core % 2
        m = dict(hg_consts[hg])
        m["x"] = np.ascontiguousarray(x[b])
        m["expander"] = expander
        in_maps.append(m)

    res = run_bass_kernel_spmd(
        nc, in_maps, core_ids=list(range(8)), trace=_trace
    )
    y = np.empty((B, C, T), np.float32)
    for b in range(B):
        y[b] = res.results[2 * b]["y"] + res.results[2 * b + 1]["y"]
    if _trace:
        _CACHE["last_results"] = res
    return y
